# revision 1
# baseline (speedup 1.0000x reference)
"""Multi-head attention (B=2, S=2048, D=2048, H=16) on 8 TRN2 NeuronCores.

Sharding: data-parallel over batch (2) x Megatron tensor-parallel over heads
(4 groups of 4 heads). Core c = 4*b + g handles batch b, heads [4g, 4g+4).
Each core computes q/k/v projections for its head slice, attention over its
4 heads, and a partial o_proj contribution; the host sums the 4 partials per
batch (the unshard step of Megatron TP) and stacks the 2 batches.

Schedule (v2, no DRAM spills):
  Phase A: k and v projections streamed over 8 x-halves; kT kept in SBUF as
  bf16 [P, 4h, S], v kept in SBUF as f32r [P, 16st, DG]. wqT prefetched.
  Phase B: per 512-query chunk (order 3,2,1,0 so the last two x-halves are
  reused from SBUF): q-projection on the fly, then per head
  {scores^T (PE), exp (ACT), attn@v (PE), denominator colsum (PE matmul
  with a ones vector, accumulated in PSUM -- keeps DVE/Pool off the inner
  loop)}, then o_proj issued at end of chunk so the Tile scheduler
  interleaves it (plus the next chunk's q-projection) into the ACT-gated
  softmax stream -- the PE never starves, which also keeps the HAM
  clock-gate at 2.4 GHz.

Matmul dtypes: q/k stored bf16 (scores matmul in bf16 -- same PE rate,
halves their SBUF footprint, LDWEIGHTS gets FWL), everything else float32r
(TF32-like, full PE rate for moving dim >= 256). Softmax statistics and
accumulations in fp32.

HW exec time (8-core SPMD): see test.py output.
"""

import math
import os

import numpy as np

import concourse.mybir as mybir
import concourse.tile as tile
from concourse import bacc
from concourse.bass_utils import run_bass_kernel_spmd

F32 = mybir.dt.float32
F32R = mybir.dt.float32r
BF16 = mybir.dt.bfloat16

B, S, D = 2, 2048, 2048
H = 16
HD = 128
G = 4
HLOC = H // G
DG = HLOC * HD
P = 128
NCORES = 8

DT = D // P            # 16 contraction tiles of 128
HALF = 256             # x streaming granule (s-columns)
NHALF = S // HALF      # 8
SCHUNK = 512
QC = S // SCHUNK       # 4 query chunks
KT = S // P            # 16 key tiles
MT = DG // P           # 4
ST_PER_CHUNK = SCHUNK // P
IC = D // SCHUNK       # 4
INV_SQRT_HD = 1.0 / math.sqrt(HD)

_cache = {}
last_run = None  # BassKernelResults of the most recent execution (for test.py)


def build():
    nc = bacc.Bacc(None, target_bir_lowering=False)

    # x packed on host as [hi, p, o, s] so each [P, 4dt, 256s] DMA reads
    # 4 KB contiguous per partition. wk/wq packed as [mt, p, o, hd] so one
    # 1 MB transfer delivers a whole stationary column block (the first
    # k-chain then only needs 3 MB of DMA before it can finish).
    xP_dr = nc.dram_tensor("xP", [NHALF * P, DT * HALF], F32R, kind="ExternalInput")
    wqT_dr = nc.dram_tensor("wqT", [MT * P, DT * HD], F32R, kind="ExternalInput")
    wkT_dr = nc.dram_tensor("wkT", [MT * P, DT * HD], F32R, kind="ExternalInput")
    wvT_dr = nc.dram_tensor("wvT", [D, DG], F32R, kind="ExternalInput")
    woT_dr = nc.dram_tensor("woT", [DG, D], F32R, kind="ExternalInput")
    out_d = nc.dram_tensor("out", [S, D], F32, kind="ExternalOutput")

    xP_v = xP_dr.rearrange("(h p) (o s) -> h p o s", p=P, s=HALF)
    wqT_v = wqT_dr.rearrange("(m p) (o h) -> m p o h", p=P, h=HD)
    wkT_v = wkT_dr.rearrange("(m p) (o h) -> m p o h", p=P, h=HD)
    wvT_v = wvT_dr.rearrange("(o p) m -> p o m", p=P)
    woT_v = woT_dr.rearrange("(o p) i -> p o i", p=P)

    with tile.TileContext(nc) as tc:
        with (
            tc.tile_pool(name="persist", bufs=1) as persist,
            tc.tile_pool(name="xs", bufs=2) as xpool,
        ):
            wqT = persist.tile([P, MT, DT, HD], F32R, tag="wqT")
            kT = persist.tile([P, HLOC, S], BF16, tag="kT")
            vv = persist.tile([P, KT, DG], F32R, tag="vv")
            ones_f = persist.tile([P, 1], F32R, tag="ones")
            ones32 = persist.tile([P, 1], F32, tag="ones32")
            nc.vector.memset(ones32[:], 1.0)
            nc.vector.tensor_copy(ones_f[:], ones32[:])

            xts = []  # phase-A half tiles; the last two are reused in phase B

            # ---------- phase A: k and v projections ----------
            with (
                tc.tile_pool(name="wA", bufs=1) as wA,
                tc.tile_pool(name="psA", bufs=1, space="PSUM") as psA,
            ):
                wkT = wA.tile([P, MT, DT, HD], F32R, tag="wkT")
                wvT = wA.tile([P, DT, DG], F32R, tag="wvT")

                for hi in range(NHALF):
                    xTh = xpool.tile([P, DT, HALF], F32R, tag="xTh")
                    xts.append(xTh)
                    # HWDGE drains FIFO in issue order: put the first
                    # k-chain's data (wk mt=0 + x half 0 = 3 MB) first,
                    # then the rest in need order.
                    if hi == 0:
                        nc.sync.dma_start(wkT[:, 0], wkT_v[0])
                    for d0 in range(0, DT, 4):
                        nc.sync.dma_start(
                            xTh[:, d0:d0 + 4], xP_v[hi, :, d0:d0 + 4, :])
                    if hi == 0:
                        for mt in range(1, MT):
                            nc.sync.dma_start(wkT[:, mt], wkT_v[mt])
                        for d0 in range(0, DT, 4):
                            nc.sync.dma_start(
                                wvT[:, d0:d0 + 4], wvT_v[:, d0:d0 + 4])
                    if hi == 3:
                        # prefetch wqT mid-phase; needed at phase-B start
                        for mt in range(MT):
                            nc.sync.dma_start(wqT[:, mt], wqT_v[mt])

                    for mt in range(MT):
                        ps = psA.tile([P, HALF], F32, tag="kps", bufs=3)
                        for dt in range(DT):
                            nc.tensor.matmul(
                                ps[:], wkT[:, mt, dt, :],
                                xTh[:, dt, :],
                                start=(dt == 0), stop=(dt == DT - 1))
                        nc.vector.tensor_copy(
                            kT[:, mt, hi * HALF:(hi + 1) * HALF], ps[:])

                    for sti in range(2):
                        st = hi * 2 + sti
                        ps = psA.tile([P, DG], F32, tag="vps", bufs=2)
                        for dt in range(DT):
                            nc.tensor.matmul(
                                ps[:], xTh[:, dt, sti * P:(sti + 1) * P],
                                wvT[:, dt, :],
                                start=(dt == 0), stop=(dt == DT - 1))
                        nc.vector.tensor_copy(vv[:, st], ps[:])

            # ---------- phase B: q-proj + attention + o_proj ----------
            with (
                tc.tile_pool(name="wB", bufs=1) as wB,
                tc.tile_pool(name="qtsp", bufs=2) as qtsp,
                tc.tile_pool(name="ctxp", bufs=2) as ctxp,
                tc.tile_pool(name="expp", bufs=6) as expp,
                tc.tile_pool(name="small", bufs=2) as small,
                tc.tile_pool(name="psB", bufs=1, space="PSUM") as psB,
            ):
                woT = wB.tile([P, MT, D], F32R, tag="woT")
                for j0 in range(MT):
                    nc.sync.dma_start(woT[:, j0:j0 + 1], woT_v[:, j0:j0 + 1])

                for qi, qc in enumerate([3, 2, 1, 0]):
                    if qi == 0:
                        xhs = [xts[6], xts[7]]
                    else:
                        xhs = []
                        for hh in range(2):
                            xb = xpool.tile([P, DT, HALF], F32R, tag="xTh")
                            hi = qc * 2 + hh
                            for d0 in range(0, DT, 4):
                                nc.sync.dma_start(
                                    xb[:, d0:d0 + 4], xP_v[hi, :, d0:d0 + 4, :])
                            xhs.append(xb)

                    qTs = qtsp.tile([P, HLOC, SCHUNK], BF16, tag="qTs")
                    for mt in range(MT):
                        for hh in range(2):
                            ps = psB.tile([P, HALF], F32, tag="qps", bufs=1)
                            for dt in range(DT):
                                nc.tensor.matmul(
                                    ps[:], wqT[:, mt, dt, :],
                                    xhs[hh][:, dt, :],
                                    start=(dt == 0), stop=(dt == DT - 1))
                            nc.vector.tensor_copy(
                                qTs[:, mt, hh * HALF:(hh + 1) * HALF], ps[:])

                    ctx = ctxp.tile([P, HLOC, SCHUNK], F32R, tag="ctx")
                    for h in range(HLOC):
                        pso = psB.tile([P, SCHUNK], F32, tag="pso", bufs=2)
                        pssum = psB.tile([1, SCHUNK], F32, tag="pssum", bufs=2)
                        for kt in range(KT):
                            pss = psB.tile([P, SCHUNK], F32, tag="pss", bufs=2)
                            nc.tensor.matmul(
                                pss[:], kT[:, h, kt * P:(kt + 1) * P],
                                qTs[:, h, :], start=True, stop=True)
                            expP = expp.tile([P, SCHUNK], F32R, tag="expP")
                            nc.scalar.activation(
                                expP[:], pss[:],
                                mybir.ActivationFunctionType.Exp,
                                scale=INV_SQRT_HD)
                            nc.tensor.matmul(
                                pso[:], vv[:, kt, h * HD:(h + 1) * HD],
                                expP[:],
                                start=(kt == 0), stop=(kt == KT - 1))
                            # denominator: colsum of exp accumulated on PE --
                            # keeps DVE/Pool out of the inner loop entirely
                            nc.tensor.matmul(
                                pssum[:], ones_f[:], expP[:],
                                start=(kt == 0), stop=(kt == KT - 1))
                        recip = small.tile([1, SCHUNK], F32, tag="recip",
                                           bufs=1)
                        nc.vector.reciprocal(recip[:], pssum[:])
                        rb = small.tile([P, SCHUNK], F32, tag="rb")
                        nc.gpsimd.partition_broadcast(rb[:], recip[:])
                        nc.vector.tensor_mul(ctx[:, h, :], pso[:], rb[:])

                    # o_proj for this chunk; scheduler interleaves it into the
                    # next chunk's softmax stream
                    for st in range(ST_PER_CHUNK):
                        stile = qc * ST_PER_CHUNK + st
                        for ic in range(IC):
                            ps = psB.tile([P, SCHUNK], F32, tag="ops", bufs=1)
                            for jt in range(MT):
                                nc.tensor.matmul(
                                    ps[:], ctx[:, jt, st * P:(st + 1) * P],
                                    woT[:, jt, ic * SCHUNK:(ic + 1) * SCHUNK],
                                    start=(jt == 0), stop=(jt == MT - 1))
                            ob = small.tile([P, SCHUNK], F32, tag="ostage")
                            nc.vector.tensor_copy(ob[:], ps[:])
                            # out stores via the (idle) gpsimd queue so x
                            # loads for the next chunk aren't stuck behind
                            # them in the sync HWDGE FIFO
                            nc.gpsimd.dma_start(
                                out_d[stile * P:(stile + 1) * P,
                                      ic * SCHUNK:(ic + 1) * SCHUNK],
                                ob[:])

    nc.finalize()
    return nc


def _round_f32r(a):
    """Round fp32 to fp32r bit patterns (round-to-nearest-even to 12 explicit
    mantissa bits, TF32-like) -- matches the hardware's own rounding."""
    u = np.ascontiguousarray(a, dtype=np.float32).view(np.uint32)
    keep = np.uint32(0xFFFFF000)
    half = np.uint32(0x7FF)
    lsb = (u >> np.uint32(12)) & np.uint32(1)
    return ((u + half + lsb) & keep).view(np.float32)


def _pack_x(xT):
    """[D, S] -> [NHALF*P, DT*HALF] blocked so device DMAs are contiguous."""
    return np.ascontiguousarray(
        xT.reshape(DT, P, NHALF, HALF).transpose(2, 1, 0, 3)
    ).reshape(NHALF * P, DT * HALF)


def _pack_w_mt(wT):
    """[D, DG] -> [MT*P, DT*HD] blocked by stationary column block mt."""
    return np.ascontiguousarray(
        wT.reshape(DT, P, MT, HD).transpose(2, 1, 0, 3)
    ).reshape(MT * P, DT * HD)


def kernel(hidden_states, wq, wk, wv, wo):
    global last_run
    if "nc" not in _cache:
        _cache["nc"] = build()
    nc = _cache["nc"]

    hidden_states = np.asarray(hidden_states, dtype=np.float32)
    wq = np.asarray(wq, dtype=np.float32)
    wk = np.asarray(wk, dtype=np.float32)
    wv = np.asarray(wv, dtype=np.float32)
    wo = np.asarray(wo, dtype=np.float32)

    xP = [_pack_x(_round_f32r(hidden_states[b].T)) for b in range(B)]
    in_maps = []
    for c in range(NCORES):
        b, g = divmod(c, G)
        sl = slice(g * DG, (g + 1) * DG)
        in_maps.append({
            "xP": xP[b],
            "wqT": _pack_w_mt(_round_f32r(wq[sl, :].T)),
            "wkT": _pack_w_mt(_round_f32r(wk[sl, :].T)),
            "wvT": _round_f32r(wv[sl, :].T),
            "woT": _round_f32r(wo[:, sl].T),
        })

    trace = os.environ.get("BASSKERNEL_TRACE", "0") == "1"
    last_run = run_bass_kernel_spmd(
        nc, in_maps, core_ids=list(range(NCORES)), trace=trace)

    out = np.empty((B, S, D), dtype=np.float32)
    for b in range(B):
        acc = None
        for g in range(G):
            part = last_run.results[b * G + g]["out"]
            acc = part.copy() if acc is None else acc + part
        out[b] = acc
    return out



# revision 3
# speedup vs baseline: 1.3381x; 1.3381x over previous
"""Multi-head attention (B=2, S=2048, D=2048, H=16) on 8 TRN2 NeuronCores.

Sharding: data-parallel over batch (2) x Megatron tensor-parallel over heads
(4 groups of 4 heads). Core c = 4*b + g handles batch b, heads [4g, 4g+4).
Host sums the 4 o_proj partials per batch and stacks the 2 batches.

Schedule (v3, all-bf16, single-pass x):
  Phase A: x streamed once in 4 granules of [P, 16dt, 512s]; per granule the
  k, v AND q projections all run (q no longer recomputed in phase B), outputs
  kept in SBUF as bf16: kT/qT [P, 4h, S], vv [P, 16st, DG]. All matmuls bf16
  (same PE rate as f32r, FWL weight loads, half the DMA/SBUF of the f32r
  baseline). Weights arrive on the scalar-engine DMA ring, x on the sync
  ring, so the first k-chain starts after ~400KB.

  Phase B per 512-query chunk, per head: scores matmuls write kt-PAIRS into a
  2-bank PSUM tile, ONE ACT exp covers the pair ([P,1024], amortizes the
  ~250ns ACT fixed cost), av matmul consumes each half (bf16). Softmax
  denominator: bf16 pair-sum tree on DVE (8+4+2 adds) then two ones-matmuls
  accumulate the partition reduction into a [128,512] PSUM tile -- every
  partition holds the sum, so reciprocal+normalize are full-width 128-lane
  DVE ops (the f32r baseline burned 54us of PE on per-kt ones-matmuls and
  3.3us/head on single-lane [1,512] reciprocals). o_proj bf16, out stored
  bf16 (host upcasts and sums partials in f32).

Per-core PE streaming floor is ~335us (qkv 164 + scores/av 109 + ones 1.7 +
o_proj 55 + DR none); ACT exp floor ~141us fully overlapped.

HW exec time (8-core SPMD): see test.py output.
"""

import math
import os

import ml_dtypes
import numpy as np

import concourse.mybir as mybir
import concourse.tile as tile
from concourse import bacc
from concourse.bass_utils import run_bass_kernel_spmd

F32 = mybir.dt.float32
BF16 = mybir.dt.bfloat16
EXP = mybir.ActivationFunctionType.Exp

B, S, D = 2, 2048, 2048
H = 16
HD = 128
G = 4
HLOC = H // G          # 4 heads per core
DG = HLOC * HD         # 512
P = 128
NCORES = 8

DT = D // P            # 16 contraction tiles
GR = 512               # x granule (s columns)
NG = S // GR           # 4
SCHUNK = 512
QC = S // SCHUNK       # 4 query chunks
KT = S // P            # 16 key tiles
MT = DG // HD          # 4 stationary column blocks per projection
IC = D // SCHUNK       # 4
ST = GR // P           # 4
INV_SQRT_HD = 1.0 / math.sqrt(HD)

_cache = {}
last_run = None  # BassKernelResults of the most recent execution (for test.py)


def build():
    nc = bacc.Bacc(None, target_bir_lowering=False)

    xP_dr = nc.dram_tensor("xP", [NG * P, DT * GR], BF16, kind="ExternalInput")
    wqT_dr = nc.dram_tensor("wqT", [MT * P, DT * HD], BF16, kind="ExternalInput")
    wkT_dr = nc.dram_tensor("wkT", [MT * P, DT * HD], BF16, kind="ExternalInput")
    wvT_dr = nc.dram_tensor("wvT", [D, DG], BF16, kind="ExternalInput")
    woT_dr = nc.dram_tensor("woT", [DG, D], BF16, kind="ExternalInput")
    out_d = nc.dram_tensor("out", [S, D], BF16, kind="ExternalOutput")

    xP_v = xP_dr.rearrange("(g p) (o s) -> g p o s", p=P, s=GR)
    wqT_v = wqT_dr.rearrange("(m p) (o h) -> m p o h", p=P, h=HD)
    wkT_v = wkT_dr.rearrange("(m p) (o h) -> m p o h", p=P, h=HD)
    wvT_v = wvT_dr.rearrange("(o p) m -> p o m", p=P)
    woT_v = woT_dr.rearrange("(o p) i -> p o i", p=P)

    with tile.TileContext(nc) as tc:
        with (
            tc.tile_pool(name="persist", bufs=1) as persist,
            tc.tile_pool(name="wA", bufs=1) as wA,
            tc.tile_pool(name="xs", bufs=2) as xpool,
            tc.tile_pool(name="expp", bufs=4) as expp,
            tc.tile_pool(name="accp", bufs=4) as accp,
            tc.tile_pool(name="ctxp", bufs=2) as ctxp,
            tc.tile_pool(name="small", bufs=2) as small,
            tc.tile_pool(name="ps", bufs=1, space="PSUM") as psp,
        ):
            kTt = persist.tile([P, HLOC, S], BF16, tag="kT")
            qTt = persist.tile([P, HLOC, S], BF16, tag="qT")
            vvt = persist.tile([P, KT, DG], BF16, tag="vv")
            wot = persist.tile([P, MT, D], BF16, tag="wo")
            onesb = persist.tile([P, P], BF16, tag="ones")
            ones32 = persist.tile([P, P], F32, tag="ones32")
            nc.vector.memset(ones32[:], 1.0)
            nc.vector.tensor_copy(onesb[:], ones32[:])

            wk_sb = wA.tile([P, MT, DT, HD], BF16, tag="wk")
            wq_sb = wA.tile([P, MT, DT, HD], BF16, tag="wq")
            wv_sb = wA.tile([P, DT, DG], BF16, tag="wv")

            # weights on the ACT DMA ring (idle until phase B), x on sync.
            # First k-chain's weights sliced fine so PE starts early.
            for d0 in range(0, DT, 4):
                nc.scalar.dma_start(wk_sb[:, 0, d0:d0 + 4], wkT_v[0, :, d0:d0 + 4])
            for mt in range(1, MT):
                nc.scalar.dma_start(wk_sb[:, mt], wkT_v[mt])
            for d0 in range(0, DT, 4):
                nc.scalar.dma_start(wv_sb[:, d0:d0 + 4], wvT_v[:, d0:d0 + 4])
            for mt in range(MT):
                nc.scalar.dma_start(wq_sb[:, mt], wqT_v[mt])
            for jt in range(MT):
                nc.scalar.dma_start(wot[:, jt:jt + 1], woT_v[:, jt:jt + 1])

            # ---------- phase A: k, v, q projections ----------
            for g in range(NG):
                xg = xpool.tile([P, DT, GR], BF16, tag="xg")
                step = 2 if g == 0 else 4
                for d0 in range(0, DT, step):
                    nc.sync.dma_start(xg[:, d0:d0 + step], xP_v[g, :, d0:d0 + step])

                for mt in range(MT):  # k projection
                    ps = psp.tile([P, GR], F32, tag="ps512", bufs=2)
                    for dt in range(DT):
                        nc.tensor.matmul(
                            ps[:], wk_sb[:, mt, dt, :], xg[:, dt, :],
                            start=(dt == 0), stop=(dt == DT - 1))
                    nc.vector.tensor_copy(
                        kTt[:, mt, g * GR:(g + 1) * GR], ps[:])

                for st in range(ST):  # v projection
                    ps = psp.tile([P, DG], F32, tag="ps512", bufs=2)
                    for dt in range(DT):
                        nc.tensor.matmul(
                            ps[:], xg[:, dt, st * P:(st + 1) * P], wv_sb[:, dt, :],
                            start=(dt == 0), stop=(dt == DT - 1))
                    nc.vector.tensor_copy(vvt[:, g * ST + st, :], ps[:])

                for mt in range(MT):  # q projection
                    ps = psp.tile([P, GR], F32, tag="ps512", bufs=2)
                    for dt in range(DT):
                        nc.tensor.matmul(
                            ps[:], wq_sb[:, mt, dt, :], xg[:, dt, :],
                            start=(dt == 0), stop=(dt == DT - 1))
                    nc.vector.tensor_copy(
                        qTt[:, mt, g * GR:(g + 1) * GR], ps[:])

            # ---------- phase B: attention + o_proj ----------
            for c in range(QC):
                ctx = ctxp.tile([P, HLOC, SCHUNK], BF16, tag="ctx")
                for h in range(HLOC):
                    pso = psp.tile([P, SCHUNK], F32, tag="pso", bufs=2)
                    # denominator bf16 pair-sum tree on DVE, issued inline so
                    # tile reuse never waits on a later-issued consumer
                    l1, l2, l3 = [], [], []
                    for kp in range(KT // 2):
                        pss = psp.tile([P, 2, SCHUNK], F32, tag="pss", bufs=2)
                        for j in range(2):
                            kt = 2 * kp + j
                            nc.tensor.matmul(
                                pss[:, j, :],
                                kTt[:, h, kt * P:(kt + 1) * P],
                                qTt[:, h, c * SCHUNK:(c + 1) * SCHUNK],
                                start=True, stop=True)
                        ex = expp.tile([P, 2, SCHUNK], BF16, tag="expP")
                        nc.scalar.activation(ex[:], pss[:], EXP,
                                             scale=INV_SQRT_HD)
                        for j in range(2):
                            kt = 2 * kp + j
                            nc.tensor.matmul(
                                pso[:], vvt[:, kt, h * HD:(h + 1) * HD],
                                ex[:, j, :],
                                start=(kt == 0), stop=(kt == KT - 1))
                        t = accp.tile([P, SCHUNK], BF16, tag="pa1", bufs=4)
                        nc.vector.tensor_add(t[:], ex[:, 0, :], ex[:, 1, :])
                        l1.append(t)
                        if kp % 2 == 1:
                            t2 = accp.tile([P, SCHUNK], BF16, tag="pa2", bufs=4)
                            nc.vector.tensor_add(t2[:], l1[-2][:], l1[-1][:])
                            l2.append(t2)
                        if kp % 4 == 3:
                            t3 = accp.tile([P, SCHUNK], BF16, tag="pa3", bufs=2)
                            nc.vector.tensor_add(t3[:], l2[-2][:], l2[-1][:])
                            l3.append(t3)

                    # two ones-matmuls reduce over partitions into [128,512]
                    # (every partition = the sum -> no broadcast needed)
                    psden = psp.tile([P, SCHUNK], F32, tag="ps512", bufs=2)
                    for i, t in enumerate(l3):
                        nc.tensor.matmul(psden[:], onesb[:], t[:],
                                         start=(i == 0), stop=(i == len(l3) - 1))
                    rb = small.tile([P, SCHUNK], F32, tag="rb")
                    nc.vector.reciprocal(rb[:], psden[:])
                    nc.vector.tensor_mul(ctx[:, h, :], pso[:], rb[:])

                # o_proj for this chunk (scheduler interleaves into the
                # ACT-gated softmax stream of the next chunk)
                for st in range(ST):
                    stile = c * ST + st
                    for ic in range(IC):
                        ps = psp.tile([P, SCHUNK], F32, tag="ps512", bufs=2)
                        for jt in range(MT):
                            nc.tensor.matmul(
                                ps[:], ctx[:, jt, st * P:(st + 1) * P],
                                wot[:, jt, ic * SCHUNK:(ic + 1) * SCHUNK],
                                start=(jt == 0), stop=(jt == MT - 1))
                        ob = small.tile([P, SCHUNK], BF16, tag="ostage", bufs=3)
                        nc.vector.tensor_copy(ob[:], ps[:])
                        nc.gpsimd.dma_start(
                            out_d[stile * P:(stile + 1) * P,
                                  ic * SCHUNK:(ic + 1) * SCHUNK],
                            ob[:])

    nc.finalize()
    return nc


def _bf16(a):
    return np.asarray(a, dtype=np.float32).astype(ml_dtypes.bfloat16)


def _pack_x(xT):
    """[D, S] bf16 -> [NG*P, DT*GR] blocked so device DMAs are contiguous."""
    return np.ascontiguousarray(
        xT.reshape(DT, P, NG, GR).transpose(2, 1, 0, 3)
    ).reshape(NG * P, DT * GR)


def _pack_w_mt(wT):
    """[D, DG] bf16 -> [MT*P, DT*HD] blocked by stationary column block."""
    return np.ascontiguousarray(
        wT.reshape(DT, P, MT, HD).transpose(2, 1, 0, 3)
    ).reshape(MT * P, DT * HD)


def kernel(hidden_states, wq, wk, wv, wo):
    global last_run
    if "nc" not in _cache:
        _cache["nc"] = build()
    nc = _cache["nc"]

    hidden_states = np.asarray(hidden_states, dtype=np.float32)
    wq = np.asarray(wq, dtype=np.float32)
    wk = np.asarray(wk, dtype=np.float32)
    wv = np.asarray(wv, dtype=np.float32)
    wo = np.asarray(wo, dtype=np.float32)

    xP = [_pack_x(_bf16(hidden_states[b].T)) for b in range(B)]
    in_maps = []
    for c in range(NCORES):
        b, g = divmod(c, G)
        sl = slice(g * DG, (g + 1) * DG)
        in_maps.append({
            "xP": xP[b],
            "wqT": _pack_w_mt(_bf16(wq[sl, :].T)),
            "wkT": _pack_w_mt(_bf16(wk[sl, :].T)),
            "wvT": _bf16(wv[sl, :].T),
            "woT": _bf16(wo[:, sl].T),
        })

    trace = os.environ.get("BASSKERNEL_TRACE", "0") == "1"
    last_run = run_bass_kernel_spmd(
        nc, in_maps, core_ids=list(range(NCORES)), trace=trace)

    out = np.empty((B, S, D), dtype=np.float32)
    for b in range(B):
        acc = None
        for g in range(G):
            part = last_run.results[b * G + g]["out"].astype(np.float32)
            acc = part if acc is None else acc + part
        out[b] = acc
    return out


# revision 7
# speedup vs baseline: 1.4522x; 1.0852x over previous
"""Multi-head attention (B=2, S=2048, D=2048, H=16) on 8 TRN2 NeuronCores.

Sharding: data-parallel over batch (2) x Megatron tensor-parallel over heads
(4 groups of 4 heads). Core c = 4*b + g handles batch b, heads [4g, 4g+4).
Host sums the 4 o_proj partials per batch and stacks the 2 batches.

Schedule (v3, all-bf16, single-pass x):
  Phase A: x streamed once in 4 granules of [P, 16dt, 512s]; per granule the
  k, v AND q projections all run (q no longer recomputed in phase B), outputs
  kept in SBUF as bf16: kT/qT [P, 4h, S], vv [P, 16st, DG]. All matmuls bf16
  (same PE rate as f32r, FWL weight loads, half the DMA/SBUF of the f32r
  baseline). Weights arrive on the scalar-engine DMA ring, x on the sync
  ring, so the first k-chain starts after ~400KB.

  Phase B per 512-query chunk, per head: scores matmuls write kt-PAIRS into a
  2-bank PSUM tile, ONE ACT exp covers the pair ([P,1024], amortizes the
  ~250ns ACT fixed cost), av matmul consumes each half (bf16). Softmax
  denominator: bf16 pair-sum tree on DVE (8+4+2 adds) then two ones-matmuls
  accumulate the partition reduction into a [128,512] PSUM tile -- every
  partition holds the sum, so reciprocal+normalize are full-width 128-lane
  DVE ops (the f32r baseline burned 54us of PE on per-kt ones-matmuls and
  3.3us/head on single-lane [1,512] reciprocals). o_proj bf16, out stored
  bf16 (host upcasts and sums partials in f32).

Per-core PE streaming floor is ~335us (qkv 164 + scores/av 109 + ones 1.7 +
o_proj 55 + DR none); ACT exp floor ~141us fully overlapped.

HW exec time (8-core SPMD): see test.py output.
"""

import math
import os

import ml_dtypes
import numpy as np

import concourse.mybir as mybir
import concourse.tile as tile
from concourse import bacc
from concourse.bass_utils import run_bass_kernel_spmd

F32 = mybir.dt.float32
BF16 = mybir.dt.bfloat16
EXP = mybir.ActivationFunctionType.Exp

B, S, D = 2, 2048, 2048
H = 16
HD = 128
G = 4
HLOC = H // G          # 4 heads per core
DG = HLOC * HD         # 512
P = 128
NCORES = 8

DT = D // P            # 16 contraction tiles
GR = 512               # x granule (s columns)
NG = S // GR           # 4
SCHUNK = 512
QC = S // SCHUNK       # 4 query chunks
KT = S // P            # 16 key tiles
MT = DG // HD          # 4 stationary column blocks per projection
IC = D // SCHUNK       # 4
ST = GR // P           # 4
INV_SQRT_HD = 1.0 / math.sqrt(HD)

_cache = {}
last_run = None  # BassKernelResults of the most recent execution (for test.py)


def build():
    nc = bacc.Bacc(None, target_bir_lowering=False)

    xP_dr = nc.dram_tensor("xP", [NG * P, DT * GR], BF16, kind="ExternalInput")
    wqT_dr = nc.dram_tensor("wqT", [MT * P, DT * HD], BF16, kind="ExternalInput")
    wkT_dr = nc.dram_tensor("wkT", [MT * P, DT * HD], BF16, kind="ExternalInput")
    wvT_dr = nc.dram_tensor("wvT", [D, DG], BF16, kind="ExternalInput")
    woT_dr = nc.dram_tensor("woT", [DG, D], BF16, kind="ExternalInput")
    out_d = nc.dram_tensor("out", [S, D], BF16, kind="ExternalOutput")

    xP_v = xP_dr.rearrange("(g p) (o s) -> g p o s", p=P, s=GR)
    wqT_v = wqT_dr.rearrange("(m p) (o h) -> m p o h", p=P, h=HD)
    wkT_v = wkT_dr.rearrange("(m p) (o h) -> m p o h", p=P, h=HD)
    wvT_v = wvT_dr.rearrange("(o p) m -> p o m", p=P)
    woT_v = woT_dr.rearrange("(o p) i -> p o i", p=P)

    with tile.TileContext(nc) as tc:
        with (
            tc.tile_pool(name="persist", bufs=1) as persist,
            tc.tile_pool(name="wA", bufs=1) as wA,
            tc.tile_pool(name="xs", bufs=2) as xpool,
            tc.tile_pool(name="expp", bufs=4) as expp,
            tc.tile_pool(name="accp", bufs=4) as accp,
            tc.tile_pool(name="ctxp", bufs=2) as ctxp,
            tc.tile_pool(name="small", bufs=2) as small,
            tc.tile_pool(name="ps", bufs=1, space="PSUM") as psp,
        ):
            kTt = persist.tile([P, HLOC, S], BF16, tag="kT")
            qTt = persist.tile([P, HLOC, S], BF16, tag="qT")
            vvt = persist.tile([P, KT, DG], BF16, tag="vv")
            wot = persist.tile([P, MT, D], BF16, tag="wo")
            onesb = persist.tile([P, P], BF16, tag="ones")
            ones32 = persist.tile([P, P], F32, tag="ones32")
            nc.vector.memset(ones32[:], 1.0)
            nc.vector.tensor_copy(onesb[:], ones32[:])

            wk_sb = wA.tile([P, MT, DT, HD], BF16, tag="wk")
            wq_sb = wA.tile([P, MT, DT, HD], BF16, tag="wq")
            wv_sb = wA.tile([P, DT, DG], BF16, tag="wv")

            # weights on the ACT DMA ring (idle until phase B), x on sync.
            # First k-chain's weights sliced fine so PE starts early.
            for d0 in range(0, DT, 2):
                nc.scalar.dma_start(wk_sb[:, 0, d0:d0 + 2], wkT_v[0, :, d0:d0 + 2])
            for mt in range(1, MT):
                nc.scalar.dma_start(wk_sb[:, mt], wkT_v[mt])
            for d0 in range(0, DT, 4):
                nc.scalar.dma_start(wv_sb[:, d0:d0 + 4], wvT_v[:, d0:d0 + 4])
            for mt in range(MT):
                nc.scalar.dma_start(wq_sb[:, mt], wqT_v[mt])
            for jt in range(MT):
                nc.scalar.dma_start(wot[:, jt:jt + 1], woT_v[:, jt:jt + 1])

            # ---------- phase A: k, v, q projections ----------
            for g in range(NG):
                xg = xpool.tile([P, DT, GR], BF16, tag="xg")
                if g == 0:
                    slices = [(d, 1) for d in range(4)] + [
                        (d, 2) for d in range(4, DT, 2)]
                else:
                    slices = [(d, 4) for d in range(0, DT, 4)]
                for d0, step in slices:
                    nc.sync.dma_start(xg[:, d0:d0 + step], xP_v[g, :, d0:d0 + step])

                for mt in range(MT):  # k projection
                    ps = psp.tile([P, GR], F32, tag="ps512", bufs=2)
                    for dt in range(DT):
                        nc.tensor.matmul(
                            ps[:], wk_sb[:, mt, dt, :], xg[:, dt, :],
                            start=(dt == 0), stop=(dt == DT - 1))
                    nc.vector.tensor_copy(
                        kTt[:, mt, g * GR:(g + 1) * GR], ps[:])

                for st in range(ST):  # v projection
                    ps = psp.tile([P, DG], F32, tag="ps512", bufs=2)
                    for dt in range(DT):
                        nc.tensor.matmul(
                            ps[:], xg[:, dt, st * P:(st + 1) * P], wv_sb[:, dt, :],
                            start=(dt == 0), stop=(dt == DT - 1))
                    nc.vector.tensor_copy(vvt[:, g * ST + st, :], ps[:])

                for mt in range(MT):  # q projection
                    ps = psp.tile([P, GR], F32, tag="ps512", bufs=2)
                    for dt in range(DT):
                        nc.tensor.matmul(
                            ps[:], wq_sb[:, mt, dt, :], xg[:, dt, :],
                            start=(dt == 0), stop=(dt == DT - 1))
                    nc.vector.tensor_copy(
                        qTt[:, mt, g * GR:(g + 1) * GR], ps[:])

            # ---------- phase B: attention + o_proj ----------
            for c in range(QC):
                ctx = ctxp.tile([P, HLOC, SCHUNK], BF16, tag="ctx")
                for h in range(HLOC):
                    pso = psp.tile([P, SCHUNK], F32, tag="pso", bufs=2)
                    # denominator bf16 pair-sum tree on DVE, issued inline so
                    # tile reuse never waits on a later-issued consumer
                    l1, l2, l3 = [], [], []
                    for kp in range(KT // 2):
                        pss = psp.tile([P, 2, SCHUNK], F32, tag="pss", bufs=2)
                        for j in range(2):
                            kt = 2 * kp + j
                            nc.tensor.matmul(
                                pss[:, j, :],
                                kTt[:, h, kt * P:(kt + 1) * P],
                                qTt[:, h, c * SCHUNK:(c + 1) * SCHUNK],
                                start=True, stop=True)
                        ex = expp.tile([P, 2, SCHUNK], BF16, tag="expP")
                        nc.scalar.activation(ex[:], pss[:], EXP,
                                             scale=INV_SQRT_HD)
                        for j in range(2):
                            kt = 2 * kp + j
                            nc.tensor.matmul(
                                pso[:], vvt[:, kt, h * HD:(h + 1) * HD],
                                ex[:, j, :],
                                start=(kt == 0), stop=(kt == KT - 1))
                        t = accp.tile([P, SCHUNK], BF16, tag="pa1", bufs=4)
                        nc.vector.tensor_add(t[:], ex[:, 0, :], ex[:, 1, :])
                        l1.append(t)
                        if kp % 2 == 1:
                            t2 = accp.tile([P, SCHUNK], BF16, tag="pa2", bufs=4)
                            nc.vector.tensor_add(t2[:], l1[-2][:], l1[-1][:])
                            l2.append(t2)
                        if kp % 4 == 3:
                            t3 = accp.tile([P, SCHUNK], BF16, tag="pa3", bufs=2)
                            nc.vector.tensor_add(t3[:], l2[-2][:], l2[-1][:])
                            l3.append(t3)

                    # two ones-matmuls reduce over partitions into [128,512]
                    # (every partition = the sum -> no broadcast needed)
                    psden = psp.tile([P, SCHUNK], F32, tag="ps512", bufs=2)
                    for i, t in enumerate(l3):
                        nc.tensor.matmul(psden[:], onesb[:], t[:],
                                         start=(i == 0), stop=(i == len(l3) - 1))
                    rb = small.tile([P, SCHUNK], F32, tag="rb")
                    # ~51 ULP is plenty for a softmax denominator; the exact
                    # DVE reciprocal costs 3.4us/call and stalled the pso
                    # PSUM pipeline
                    nc.vector.reciprocal_approx_fast(rb[:], psden[:])
                    nc.vector.tensor_mul(ctx[:, h, :], pso[:], rb[:])

                # o_proj for this chunk (scheduler interleaves into the
                # ACT-gated softmax stream of the next chunk)
                for st in range(ST):
                    stile = c * ST + st
                    for ic in range(IC):
                        ps = psp.tile([P, SCHUNK], F32, tag="ps512", bufs=2)
                        for jt in range(MT):
                            nc.tensor.matmul(
                                ps[:], ctx[:, jt, st * P:(st + 1) * P],
                                wot[:, jt, ic * SCHUNK:(ic + 1) * SCHUNK],
                                start=(jt == 0), stop=(jt == MT - 1))
                        ob = small.tile([P, SCHUNK], BF16, tag="ostage", bufs=3)
                        nc.vector.tensor_copy(ob[:], ps[:])
                        # sync HWDGE ring is idle in phase B (x loads done)
                        # and avoids the multi-us SWDGE drain at teardown
                        nc.sync.dma_start(
                            out_d[stile * P:(stile + 1) * P,
                                  ic * SCHUNK:(ic + 1) * SCHUNK],
                            ob[:])

    nc.finalize()
    return nc


def _bf16(a):
    return np.asarray(a, dtype=np.float32).astype(ml_dtypes.bfloat16)


def _pack_x(xT):
    """[D, S] bf16 -> [NG*P, DT*GR] blocked so device DMAs are contiguous."""
    return np.ascontiguousarray(
        xT.reshape(DT, P, NG, GR).transpose(2, 1, 0, 3)
    ).reshape(NG * P, DT * GR)


def _pack_w_mt(wT):
    """[D, DG] bf16 -> [MT*P, DT*HD] blocked by stationary column block."""
    return np.ascontiguousarray(
        wT.reshape(DT, P, MT, HD).transpose(2, 1, 0, 3)
    ).reshape(MT * P, DT * HD)


def kernel(hidden_states, wq, wk, wv, wo):
    global last_run
    if "nc" not in _cache:
        _cache["nc"] = build()
    nc = _cache["nc"]

    hidden_states = np.asarray(hidden_states, dtype=np.float32)
    wq = np.asarray(wq, dtype=np.float32)
    wk = np.asarray(wk, dtype=np.float32)
    wv = np.asarray(wv, dtype=np.float32)
    wo = np.asarray(wo, dtype=np.float32)

    xP = [_pack_x(_bf16(hidden_states[b].T)) for b in range(B)]
    in_maps = []
    for c in range(NCORES):
        b, g = divmod(c, G)
        sl = slice(g * DG, (g + 1) * DG)
        in_maps.append({
            "xP": xP[b],
            "wqT": _pack_w_mt(_bf16(wq[sl, :].T)),
            "wkT": _pack_w_mt(_bf16(wk[sl, :].T)),
            "wvT": _bf16(wv[sl, :].T),
            "woT": _bf16(wo[:, sl].T),
        })

    trace = os.environ.get("BASSKERNEL_TRACE", "0") == "1"
    last_run = run_bass_kernel_spmd(
        nc, in_maps, core_ids=list(range(NCORES)), trace=trace)

    out = np.empty((B, S, D), dtype=np.float32)
    for b in range(B):
        acc = None
        for g in range(G):
            part = last_run.results[b * G + g]["out"].astype(np.float32)
            acc = part if acc is None else acc + part
        out[b] = acc
    return out


# revision 9
# speedup vs baseline: 1.4563x; 1.0028x over previous
"""Multi-head attention (B=2, S=2048, D=2048, H=16) on 8 TRN2 NeuronCores.

Sharding: data-parallel over batch (2) x Megatron tensor-parallel over heads
(4 groups of 4 heads). Core c = 4*b + g handles batch b, heads [4g, 4g+4).
Host sums the 4 o_proj partials per batch and stacks the 2 batches.

Schedule (v3, all-bf16, single-pass x):
  Phase A: x streamed once in 4 granules of [P, 16dt, 512s]; per granule the
  k, v AND q projections all run (q no longer recomputed in phase B), outputs
  kept in SBUF as bf16: kT/qT [P, 4h, S], vv [P, 16st, DG]. All matmuls bf16
  (same PE rate as f32r, FWL weight loads, half the DMA/SBUF of the f32r
  baseline). Weights arrive on the scalar-engine DMA ring, x on the sync
  ring, so the first k-chain starts after ~400KB.

  Phase B per 512-query chunk, per head: scores matmuls write kt-PAIRS into a
  2-bank PSUM tile, ONE ACT exp covers the pair ([P,1024], amortizes the
  ~250ns ACT fixed cost), av matmul consumes each half (bf16). Softmax
  denominator: bf16 pair-sum tree on DVE (8+4+2 adds) then two ones-matmuls
  accumulate the partition reduction into a [128,512] PSUM tile -- every
  partition holds the sum, so reciprocal+normalize are full-width 128-lane
  DVE ops (the f32r baseline burned 54us of PE on per-kt ones-matmuls and
  3.3us/head on single-lane [1,512] reciprocals). o_proj bf16, out stored
  bf16 (host upcasts and sums partials in f32).

Per-core PE streaming floor is ~335us (qkv 164 + scores/av 109 + ones 1.7 +
o_proj 55 + DR none); ACT exp floor ~141us fully overlapped.

HW exec time (8-core SPMD): see test.py output.
"""

import math
import os

import ml_dtypes
import numpy as np

import concourse.mybir as mybir
import concourse.tile as tile
from concourse import bacc
from concourse.bass_utils import run_bass_kernel_spmd

F32 = mybir.dt.float32
BF16 = mybir.dt.bfloat16
EXP = mybir.ActivationFunctionType.Exp

B, S, D = 2, 2048, 2048
H = 16
HD = 128
G = 4
HLOC = H // G          # 4 heads per core
DG = HLOC * HD         # 512
P = 128
NCORES = 8

DT = D // P            # 16 contraction tiles
GR = 512               # x granule (s columns)
NG = S // GR           # 4
SCHUNK = 512
QC = S // SCHUNK       # 4 query chunks
KT = S // P            # 16 key tiles
MT = DG // HD          # 4 stationary column blocks per projection
IC = D // SCHUNK       # 4
ST = GR // P           # 4
INV_SQRT_HD = 1.0 / math.sqrt(HD)

_cache = {}
last_run = None  # BassKernelResults of the most recent execution (for test.py)


def build():
    nc = bacc.Bacc(None, target_bir_lowering=False)

    xP_dr = nc.dram_tensor("xP", [NG * P, DT * GR], BF16, kind="ExternalInput")
    wqT_dr = nc.dram_tensor("wqT", [MT * P, DT * HD], BF16, kind="ExternalInput")
    wkT_dr = nc.dram_tensor("wkT", [MT * P, DT * HD], BF16, kind="ExternalInput")
    wvT_dr = nc.dram_tensor("wvT", [D, DG], BF16, kind="ExternalInput")
    woT_dr = nc.dram_tensor("woT", [DG, D], BF16, kind="ExternalInput")
    out_d = nc.dram_tensor("out", [S, D], BF16, kind="ExternalOutput")

    xP_v = xP_dr.rearrange("(g p) (o s) -> g p o s", p=P, s=GR)
    wqT_v = wqT_dr.rearrange("(m p) (o h) -> m p o h", p=P, h=HD)
    wkT_v = wkT_dr.rearrange("(m p) (o h) -> m p o h", p=P, h=HD)
    wvT_v = wvT_dr.rearrange("(o p) m -> p o m", p=P)
    woT_v = woT_dr.rearrange("(o p) i -> p o i", p=P)

    with tile.TileContext(nc) as tc:
        with (
            tc.tile_pool(name="persist", bufs=1) as persist,
            tc.tile_pool(name="wA", bufs=1) as wA,
            tc.tile_pool(name="xs", bufs=2) as xpool,
            tc.tile_pool(name="expp", bufs=4) as expp,
            tc.tile_pool(name="accp", bufs=4) as accp,
            tc.tile_pool(name="ctxp", bufs=2) as ctxp,
            tc.tile_pool(name="small", bufs=2) as small,
            tc.tile_pool(name="ps", bufs=1, space="PSUM") as psp,
        ):
            kTt = persist.tile([P, HLOC, S], BF16, tag="kT")
            qTt = persist.tile([P, HLOC, S], BF16, tag="qT")
            vvt = persist.tile([P, KT, DG], BF16, tag="vv")
            wot = persist.tile([P, MT, D], BF16, tag="wo")
            onesb = persist.tile([P, P], BF16, tag="ones")
            ones32 = persist.tile([P, P], F32, tag="ones32")
            nc.vector.memset(ones32[:], 1.0)
            nc.vector.tensor_copy(onesb[:], ones32[:])

            # ~4us of tiny matmuls bridging the initial DMA wait: keeps the
            # PE-HAM activity window busy so the real chains start at 2.4GHz
            # instead of paying ~3.4us of half-clock warmup mid-phase-A
            warm = psp.tile([P, GR], F32, tag="ps512", bufs=2)
            for _ in range(56):
                nc.tensor.matmul(warm[0:64, 0:64], onesb[:, 0:64],
                                 onesb[:, 0:64], start=True, stop=True)

            wk_sb = wA.tile([P, MT, DT, HD], BF16, tag="wk")
            wq_sb = wA.tile([P, MT, DT, HD], BF16, tag="wq")
            wv_sb = wA.tile([P, DT, DG], BF16, tag="wv")

            # weights on the ACT DMA ring (idle until phase B), x on sync.
            # First k-chain's weights sliced fine so PE starts early.
            for d0 in range(0, DT, 2):
                nc.scalar.dma_start(wk_sb[:, 0, d0:d0 + 2], wkT_v[0, :, d0:d0 + 2])
            for mt in range(1, MT):
                nc.scalar.dma_start(wk_sb[:, mt], wkT_v[mt])
            for d0 in range(0, DT, 4):
                nc.scalar.dma_start(wv_sb[:, d0:d0 + 4], wvT_v[:, d0:d0 + 4])
            for mt in range(MT):
                nc.scalar.dma_start(wq_sb[:, mt], wqT_v[mt])
            for jt in range(MT):
                nc.scalar.dma_start(wot[:, jt:jt + 1], woT_v[:, jt:jt + 1])

            # ---------- phase A: k, v, q projections ----------
            for g in range(NG):
                xg = xpool.tile([P, DT, GR], BF16, tag="xg")
                if g == 0:
                    slices = [(d, 1) for d in range(4)] + [
                        (d, 2) for d in range(4, DT, 2)]
                else:
                    slices = [(d, 4) for d in range(0, DT, 4)]
                for d0, step in slices:
                    nc.sync.dma_start(xg[:, d0:d0 + step], xP_v[g, :, d0:d0 + step])

                for mt in range(MT):  # k projection
                    ps = psp.tile([P, GR], F32, tag="ps512", bufs=2)
                    for dt in range(DT):
                        nc.tensor.matmul(
                            ps[:], wk_sb[:, mt, dt, :], xg[:, dt, :],
                            start=(dt == 0), stop=(dt == DT - 1))
                    nc.vector.tensor_copy(
                        kTt[:, mt, g * GR:(g + 1) * GR], ps[:])

                for st in range(ST):  # v projection
                    ps = psp.tile([P, DG], F32, tag="ps512", bufs=2)
                    for dt in range(DT):
                        nc.tensor.matmul(
                            ps[:], xg[:, dt, st * P:(st + 1) * P], wv_sb[:, dt, :],
                            start=(dt == 0), stop=(dt == DT - 1))
                    nc.vector.tensor_copy(vvt[:, g * ST + st, :], ps[:])

                for mt in range(MT):  # q projection
                    ps = psp.tile([P, GR], F32, tag="ps512", bufs=2)
                    for dt in range(DT):
                        nc.tensor.matmul(
                            ps[:], wq_sb[:, mt, dt, :], xg[:, dt, :],
                            start=(dt == 0), stop=(dt == DT - 1))
                    nc.vector.tensor_copy(
                        qTt[:, mt, g * GR:(g + 1) * GR], ps[:])

            # ---------- phase B: attention + o_proj ----------
            for c in range(QC):
                ctx = ctxp.tile([P, HLOC, SCHUNK], BF16, tag="ctx")
                for h in range(HLOC):
                    pso = psp.tile([P, SCHUNK], F32, tag="pso", bufs=2)
                    # denominator bf16 pair-sum tree on DVE, issued inline so
                    # tile reuse never waits on a later-issued consumer
                    l1, l2, l3 = [], [], []
                    for kp in range(KT // 2):
                        pss = psp.tile([P, 2, SCHUNK], F32, tag="pss", bufs=2)
                        for j in range(2):
                            kt = 2 * kp + j
                            nc.tensor.matmul(
                                pss[:, j, :],
                                kTt[:, h, kt * P:(kt + 1) * P],
                                qTt[:, h, c * SCHUNK:(c + 1) * SCHUNK],
                                start=True, stop=True)
                        ex = expp.tile([P, 2, SCHUNK], BF16, tag="expP",
                                       bufs=6)
                        nc.scalar.activation(ex[:], pss[:], EXP,
                                             scale=INV_SQRT_HD)
                        for j in range(2):
                            kt = 2 * kp + j
                            nc.tensor.matmul(
                                pso[:], vvt[:, kt, h * HD:(h + 1) * HD],
                                ex[:, j, :],
                                start=(kt == 0), stop=(kt == KT - 1))
                        t = accp.tile([P, SCHUNK], BF16, tag="pa1", bufs=4)
                        nc.vector.tensor_add(t[:], ex[:, 0, :], ex[:, 1, :])
                        l1.append(t)
                        if kp % 2 == 1:
                            t2 = accp.tile([P, SCHUNK], BF16, tag="pa2", bufs=4)
                            nc.vector.tensor_add(t2[:], l1[-2][:], l1[-1][:])
                            l2.append(t2)
                        if kp % 4 == 3:
                            t3 = accp.tile([P, SCHUNK], BF16, tag="pa3", bufs=2)
                            nc.vector.tensor_add(t3[:], l2[-2][:], l2[-1][:])
                            l3.append(t3)

                    # two ones-matmuls reduce over partitions into [128,512]
                    # (every partition = the sum -> no broadcast needed)
                    psden = psp.tile([P, SCHUNK], F32, tag="ps512", bufs=2)
                    for i, t in enumerate(l3):
                        nc.tensor.matmul(psden[:], onesb[:], t[:],
                                         start=(i == 0), stop=(i == len(l3) - 1))
                    rb = small.tile([P, SCHUNK], F32, tag="rb")
                    # ~51 ULP is plenty for a softmax denominator; the exact
                    # DVE reciprocal costs 3.4us/call and stalled the pso
                    # PSUM pipeline
                    nc.vector.reciprocal_approx_fast(rb[:], psden[:])
                    nc.vector.tensor_mul(ctx[:, h, :], pso[:], rb[:])

                # o_proj for this chunk (scheduler interleaves into the
                # ACT-gated softmax stream of the next chunk)
                for st in range(ST):
                    stile = c * ST + st
                    for ic in range(IC):
                        ps = psp.tile([P, SCHUNK], F32, tag="ps512", bufs=2)
                        for jt in range(MT):
                            nc.tensor.matmul(
                                ps[:], ctx[:, jt, st * P:(st + 1) * P],
                                wot[:, jt, ic * SCHUNK:(ic + 1) * SCHUNK],
                                start=(jt == 0), stop=(jt == MT - 1))
                        ob = small.tile([P, SCHUNK], BF16, tag="ostage", bufs=3)
                        nc.vector.tensor_copy(ob[:], ps[:])
                        # sync HWDGE ring is idle in phase B (x loads done)
                        # and avoids the multi-us SWDGE drain at teardown
                        nc.sync.dma_start(
                            out_d[stile * P:(stile + 1) * P,
                                  ic * SCHUNK:(ic + 1) * SCHUNK],
                            ob[:])

    nc.finalize()
    return nc


def _bf16(a):
    return np.asarray(a, dtype=np.float32).astype(ml_dtypes.bfloat16)


def _pack_x(xT):
    """[D, S] bf16 -> [NG*P, DT*GR] blocked so device DMAs are contiguous."""
    return np.ascontiguousarray(
        xT.reshape(DT, P, NG, GR).transpose(2, 1, 0, 3)
    ).reshape(NG * P, DT * GR)


def _pack_w_mt(wT):
    """[D, DG] bf16 -> [MT*P, DT*HD] blocked by stationary column block."""
    return np.ascontiguousarray(
        wT.reshape(DT, P, MT, HD).transpose(2, 1, 0, 3)
    ).reshape(MT * P, DT * HD)


def kernel(hidden_states, wq, wk, wv, wo):
    global last_run
    if "nc" not in _cache:
        _cache["nc"] = build()
    nc = _cache["nc"]

    hidden_states = np.asarray(hidden_states, dtype=np.float32)
    wq = np.asarray(wq, dtype=np.float32)
    wk = np.asarray(wk, dtype=np.float32)
    wv = np.asarray(wv, dtype=np.float32)
    wo = np.asarray(wo, dtype=np.float32)

    xP = [_pack_x(_bf16(hidden_states[b].T)) for b in range(B)]
    in_maps = []
    for c in range(NCORES):
        b, g = divmod(c, G)
        sl = slice(g * DG, (g + 1) * DG)
        in_maps.append({
            "xP": xP[b],
            "wqT": _pack_w_mt(_bf16(wq[sl, :].T)),
            "wkT": _pack_w_mt(_bf16(wk[sl, :].T)),
            "wvT": _bf16(wv[sl, :].T),
            "woT": _bf16(wo[:, sl].T),
        })

    trace = os.environ.get("BASSKERNEL_TRACE", "0") == "1"
    last_run = run_bass_kernel_spmd(
        nc, in_maps, core_ids=list(range(NCORES)), trace=trace)

    out = np.empty((B, S, D), dtype=np.float32)
    for b in range(B):
        acc = None
        for g in range(G):
            part = last_run.results[b * G + g]["out"].astype(np.float32)
            acc = part if acc is None else acc + part
        out[b] = acc
    return out


# revision 14
# speedup vs baseline: 1.4617x; 1.0037x over previous
"""Multi-head attention (B=2, S=2048, D=2048, H=16) on 8 TRN2 NeuronCores.

Sharding: data-parallel over batch (2) x Megatron tensor-parallel over heads
(4 groups of 4 heads). Core c = 4*b + g handles batch b, heads [4g, 4g+4).
Host sums the 4 o_proj partials per batch and stacks the 2 batches.

Schedule (v3, all-bf16, single-pass x):
  Phase A: x streamed once in 4 granules of [P, 16dt, 512s]; per granule the
  k, v AND q projections all run (q no longer recomputed in phase B), outputs
  kept in SBUF as bf16: kT/qT [P, 4h, S], vv [P, 16st, DG]. All matmuls bf16
  (same PE rate as f32r, FWL weight loads, half the DMA/SBUF of the f32r
  baseline). Weights arrive on the scalar-engine DMA ring, x on the sync
  ring, so the first k-chain starts after ~400KB.

  Phase B per 512-query chunk, per head: scores matmuls write kt-PAIRS into a
  2-bank PSUM tile, ONE ACT exp covers the pair ([P,1024], amortizes the
  ~250ns ACT fixed cost), av matmul consumes each half (bf16). Softmax
  denominator: bf16 pair-sum tree on DVE (8+4+2 adds) then two ones-matmuls
  accumulate the partition reduction into a [128,512] PSUM tile -- every
  partition holds the sum, so reciprocal+normalize are full-width 128-lane
  DVE ops (the f32r baseline burned 54us of PE on per-kt ones-matmuls and
  3.3us/head on single-lane [1,512] reciprocals). o_proj bf16, out stored
  bf16 (host upcasts and sums partials in f32).

Per-core PE streaming floor is ~335us (qkv 164 + scores/av 109 + ones 1.7 +
o_proj 55 + DR none); ACT exp floor ~141us fully overlapped.

HW exec time (8-core SPMD): see test.py output.
"""

import math
import os

import ml_dtypes
import numpy as np

import concourse.mybir as mybir
import concourse.tile as tile
from concourse import bacc
from concourse.bass_utils import run_bass_kernel_spmd

F32 = mybir.dt.float32
BF16 = mybir.dt.bfloat16
EXP = mybir.ActivationFunctionType.Exp

B, S, D = 2, 2048, 2048
H = 16
HD = 128
G = 4
HLOC = H // G          # 4 heads per core
DG = HLOC * HD         # 512
P = 128
NCORES = 8

DT = D // P            # 16 contraction tiles
GR = 512               # x granule (s columns)
NG = S // GR           # 4
SCHUNK = 512
QC = S // SCHUNK       # 4 query chunks
KT = S // P            # 16 key tiles
MT = DG // HD          # 4 stationary column blocks per projection
IC = D // SCHUNK       # 4
ST = GR // P           # 4
INV_SQRT_HD = 1.0 / math.sqrt(HD)

_cache = {}
last_run = None  # BassKernelResults of the most recent execution (for test.py)


def build():
    nc = bacc.Bacc(None, target_bir_lowering=False)

    xP_dr = nc.dram_tensor("xP", [NG * P, DT * GR], BF16, kind="ExternalInput")
    wqT_dr = nc.dram_tensor("wqT", [MT * P, DT * HD], BF16, kind="ExternalInput")
    wkT_dr = nc.dram_tensor("wkT", [MT * P, DT * HD], BF16, kind="ExternalInput")
    wvT_dr = nc.dram_tensor("wvT", [D, DG], BF16, kind="ExternalInput")
    woT_dr = nc.dram_tensor("woT", [DG, D], BF16, kind="ExternalInput")
    out_d = nc.dram_tensor("out", [S, D], BF16, kind="ExternalOutput")

    xP_v = xP_dr.rearrange("(g p) (o s) -> g p o s", p=P, s=GR)
    wqT_v = wqT_dr.rearrange("(m p) (o h) -> m p o h", p=P, h=HD)
    wkT_v = wkT_dr.rearrange("(m p) (o h) -> m p o h", p=P, h=HD)
    wvT_v = wvT_dr.rearrange("(o p) m -> p o m", p=P)
    woT_v = woT_dr.rearrange("(o p) i -> p o i", p=P)

    with tile.TileContext(nc) as tc:
        with (
            tc.tile_pool(name="persist", bufs=1) as persist,
            tc.tile_pool(name="wA", bufs=1) as wA,
            tc.tile_pool(name="xs", bufs=2) as xpool,
            tc.tile_pool(name="expp", bufs=4) as expp,
            tc.tile_pool(name="accp", bufs=4) as accp,
            tc.tile_pool(name="ctxp", bufs=2) as ctxp,
            tc.tile_pool(name="small", bufs=2) as small,
            tc.tile_pool(name="ps", bufs=1, space="PSUM") as psp,
        ):
            kTt = persist.tile([P, HLOC, S], BF16, tag="kT")
            qTt = persist.tile([P, HLOC, S], BF16, tag="qT")
            vvt = persist.tile([P, KT, DG], BF16, tag="vv")
            wot = persist.tile([P, MT, D], BF16, tag="wo")
            onesb = persist.tile([P, P], BF16, tag="ones")
            ones32 = persist.tile([P, P], F32, tag="ones32")
            nc.vector.memset(ones32[:], 1.0)
            nc.vector.tensor_copy(onesb[:], ones32[:])

            # ~4us of tiny matmuls bridging the initial DMA wait: keeps the
            # PE-HAM activity window busy so the real chains start at 2.4GHz
            # instead of paying ~3.4us of half-clock warmup mid-phase-A
            warm = psp.tile([P, GR], F32, tag="ps512", bufs=2)
            for _ in range(56):
                nc.tensor.matmul(warm[0:64, 0:64], onesb[:, 0:64],
                                 onesb[:, 0:64], start=True, stop=True)

            wk_sb = wA.tile([P, MT, DT, HD], BF16, tag="wk")
            wq_sb = wA.tile([P, MT, DT, HD], BF16, tag="wq")
            wv_sb = wA.tile([P, DT, DG], BF16, tag="wv")

            # weights on the ACT DMA ring (idle until phase B), x on sync.
            # First k-chain's weights sliced fine so PE starts early.
            xg0 = xpool.tile([P, DT, GR], BF16, tag="xg")
            for d0 in range(0, DT, 2):
                nc.scalar.dma_start(wk_sb[:, 0, d0:d0 + 2], wkT_v[0, :, d0:d0 + 2])
            # granule 0's upper half rides the scalar ring (one ring's issue
            # rate can't land 2MB before the first k chains drain it)
            for d0 in range(8, DT, 4):
                nc.scalar.dma_start(xg0[:, d0:d0 + 4], xP_v[0, :, d0:d0 + 4])
            for mt in range(1, MT):
                nc.scalar.dma_start(wk_sb[:, mt], wkT_v[mt])
            for d0 in range(0, DT, 4):
                nc.scalar.dma_start(wv_sb[:, d0:d0 + 4], wvT_v[:, d0:d0 + 4])
            for mt in range(MT):
                nc.scalar.dma_start(wq_sb[:, mt], wqT_v[mt])
            for jt in range(MT):
                nc.scalar.dma_start(wot[:, jt:jt + 1], woT_v[:, jt:jt + 1])

            # ---------- phase A: k, v, q projections ----------
            for g in range(NG):
                if g == 0:
                    xg = xg0
                    for d0 in range(4):
                        nc.sync.dma_start(xg[:, d0:d0 + 1], xP_v[g, :, d0:d0 + 1])
                    for d0 in range(4, 8, 2):
                        nc.sync.dma_start(xg[:, d0:d0 + 2], xP_v[g, :, d0:d0 + 2])
                else:
                    xg = xpool.tile([P, DT, GR], BF16, tag="xg")
                    for d0 in range(0, DT, 4):
                        nc.sync.dma_start(xg[:, d0:d0 + 4], xP_v[g, :, d0:d0 + 4])

                for mt in range(MT):  # k projection
                    ps = psp.tile([P, GR], F32, tag="ps512", bufs=2)
                    for dt in range(DT):
                        nc.tensor.matmul(
                            ps[:], wk_sb[:, mt, dt, :], xg[:, dt, :],
                            start=(dt == 0), stop=(dt == DT - 1))
                    nc.vector.tensor_copy(
                        kTt[:, mt, g * GR:(g + 1) * GR], ps[:])

                for st in range(ST):  # v projection
                    ps = psp.tile([P, DG], F32, tag="ps512", bufs=2)
                    for dt in range(DT):
                        nc.tensor.matmul(
                            ps[:], xg[:, dt, st * P:(st + 1) * P], wv_sb[:, dt, :],
                            start=(dt == 0), stop=(dt == DT - 1))
                    nc.vector.tensor_copy(vvt[:, g * ST + st, :], ps[:])

                for mt in range(MT):  # q projection
                    ps = psp.tile([P, GR], F32, tag="ps512", bufs=2)
                    for dt in range(DT):
                        nc.tensor.matmul(
                            ps[:], wq_sb[:, mt, dt, :], xg[:, dt, :],
                            start=(dt == 0), stop=(dt == DT - 1))
                    nc.vector.tensor_copy(
                        qTt[:, mt, g * GR:(g + 1) * GR], ps[:])

            # ---------- phase B: attention + o_proj ----------
            def oproj_group(ctx_c, c, st, ic):
                stile = c * ST + st
                ps = psp.tile([P, SCHUNK], F32, tag="ps512", bufs=2)
                for jt in range(MT):
                    nc.tensor.matmul(
                        ps[:], ctx_c[:, jt, st * P:(st + 1) * P],
                        wot[:, jt, ic * SCHUNK:(ic + 1) * SCHUNK],
                        start=(jt == 0), stop=(jt == MT - 1))
                ob = small.tile([P, SCHUNK], BF16, tag="ostage", bufs=3)
                nc.vector.tensor_copy(ob[:], ps[:])
                # sync HWDGE ring is idle in phase B (x loads done) and
                # avoids the multi-us SWDGE drain at teardown
                nc.sync.dma_start(
                    out_d[stile * P:(stile + 1) * P,
                          ic * SCHUNK:(ic + 1) * SCHUNK],
                    ob[:])

            # o_proj of chunk c-1 is issued interleaved between the heads of
            # chunk c: its ps512 tiles then rotate between the psden tiles
            # instead of queueing behind all four of them, and its matmuls
            # give PE filler work at every head boundary
            pending = []  # (ctx, c, st, ic) groups not yet issued
            for c in range(QC):
                ctx = ctxp.tile([P, HLOC, SCHUNK], BF16, tag="ctx")
                for h in range(HLOC):
                    pso = psp.tile([P, SCHUNK], F32, tag="pso", bufs=2)
                    # denominator bf16 pair-sum tree on DVE, issued inline so
                    # tile reuse never waits on a later-issued consumer
                    l1, l2, l3 = [], [], []
                    for kp in range(KT // 2):
                        pss = psp.tile([P, 2, SCHUNK], F32, tag="pss", bufs=2)
                        for j in range(2):
                            kt = 2 * kp + j
                            nc.tensor.matmul(
                                pss[:, j, :],
                                kTt[:, h, kt * P:(kt + 1) * P],
                                qTt[:, h, c * SCHUNK:(c + 1) * SCHUNK],
                                start=True, stop=True)
                        ex = expp.tile([P, 2, SCHUNK], BF16, tag="expP",
                                       bufs=6)
                        nc.scalar.activation(ex[:], pss[:], EXP,
                                             scale=INV_SQRT_HD)
                        for j in range(2):
                            kt = 2 * kp + j
                            nc.tensor.matmul(
                                pso[:], vvt[:, kt, h * HD:(h + 1) * HD],
                                ex[:, j, :],
                                start=(kt == 0), stop=(kt == KT - 1))
                        t = accp.tile([P, SCHUNK], BF16, tag="pa1", bufs=4)
                        nc.vector.tensor_add(t[:], ex[:, 0, :], ex[:, 1, :])
                        l1.append(t)
                        if kp % 2 == 1:
                            t2 = accp.tile([P, SCHUNK], BF16, tag="pa2", bufs=4)
                            nc.vector.tensor_add(t2[:], l1[-2][:], l1[-1][:])
                            l2.append(t2)
                        if kp % 4 == 3:
                            t3 = accp.tile([P, SCHUNK], BF16, tag="pa3", bufs=2)
                            nc.vector.tensor_add(t3[:], l2[-2][:], l2[-1][:])
                            l3.append(t3)

                    # two ones-matmuls reduce over partitions into [128,512]
                    # (every partition = the sum -> no broadcast needed)
                    psden = psp.tile([P, SCHUNK], F32, tag="ps512", bufs=2)
                    for i, t in enumerate(l3):
                        nc.tensor.matmul(psden[:], onesb[:], t[:],
                                         start=(i == 0), stop=(i == len(l3) - 1))
                    rb = small.tile([P, SCHUNK], F32, tag="rb")
                    # ~51 ULP is plenty for a softmax denominator; the exact
                    # DVE reciprocal costs 3.4us/call and stalled the pso
                    # PSUM pipeline
                    nc.vector.reciprocal_approx_fast(rb[:], psden[:])
                    nc.vector.tensor_mul(ctx[:, h, :], pso[:], rb[:])

                    for _ in range(min(4, len(pending))):
                        oproj_group(*pending.pop(0))

                for st in range(ST):
                    for ic in range(IC):
                        pending.append((ctx, c, st, ic))
            while pending:
                oproj_group(*pending.pop(0))

    nc.finalize()
    return nc


def _bf16(a):
    return np.asarray(a, dtype=np.float32).astype(ml_dtypes.bfloat16)


def _pack_x(xT):
    """[D, S] bf16 -> [NG*P, DT*GR] blocked so device DMAs are contiguous."""
    return np.ascontiguousarray(
        xT.reshape(DT, P, NG, GR).transpose(2, 1, 0, 3)
    ).reshape(NG * P, DT * GR)


def _pack_w_mt(wT):
    """[D, DG] bf16 -> [MT*P, DT*HD] blocked by stationary column block."""
    return np.ascontiguousarray(
        wT.reshape(DT, P, MT, HD).transpose(2, 1, 0, 3)
    ).reshape(MT * P, DT * HD)


def kernel(hidden_states, wq, wk, wv, wo):
    global last_run
    if "nc" not in _cache:
        _cache["nc"] = build()
    nc = _cache["nc"]

    hidden_states = np.asarray(hidden_states, dtype=np.float32)
    wq = np.asarray(wq, dtype=np.float32)
    wk = np.asarray(wk, dtype=np.float32)
    wv = np.asarray(wv, dtype=np.float32)
    wo = np.asarray(wo, dtype=np.float32)

    xP = [_pack_x(_bf16(hidden_states[b].T)) for b in range(B)]
    in_maps = []
    for c in range(NCORES):
        b, g = divmod(c, G)
        sl = slice(g * DG, (g + 1) * DG)
        in_maps.append({
            "xP": xP[b],
            "wqT": _pack_w_mt(_bf16(wq[sl, :].T)),
            "wkT": _pack_w_mt(_bf16(wk[sl, :].T)),
            "wvT": _bf16(wv[sl, :].T),
            "woT": _bf16(wo[:, sl].T),
        })

    trace = os.environ.get("BASSKERNEL_TRACE", "0") == "1"
    last_run = run_bass_kernel_spmd(
        nc, in_maps, core_ids=list(range(NCORES)), trace=trace)

    out = np.empty((B, S, D), dtype=np.float32)
    for b in range(B):
        acc = None
        for g in range(G):
            part = last_run.results[b * G + g]["out"].astype(np.float32)
            acc = part if acc is None else acc + part
        out[b] = acc
    return out


# revision 17
# speedup vs baseline: 1.5205x; 1.0403x over previous
"""Multi-head attention (B=2, S=2048, D=2048, H=16) on 8 TRN2 NeuronCores.

Sharding: data-parallel over batch (2) x Megatron tensor-parallel over heads
(4 groups of 4 heads). Core c = 4*b + g handles batch b, heads [4g, 4g+4).
Host sums the 4 o_proj partials per batch and stacks the 2 batches.

Schedule (v3, all-bf16, single-pass x):
  Phase A: x streamed once in 4 granules of [P, 16dt, 512s]; per granule the
  k, v AND q projections all run (q no longer recomputed in phase B), outputs
  kept in SBUF as bf16: kT/qT [P, 4h, S], vv [P, 16st, DG]. All matmuls bf16
  (same PE rate as f32r, FWL weight loads, half the DMA/SBUF of the f32r
  baseline). Weights arrive on the scalar-engine DMA ring, x on the sync
  ring, so the first k-chain starts after ~400KB.

  Phase B per 512-query chunk, per head: scores matmuls write kt-PAIRS into a
  2-bank PSUM tile, ONE ACT exp covers the pair ([P,1024], amortizes the
  ~250ns ACT fixed cost), av matmul consumes each half (bf16). Softmax
  denominator: bf16 pair-sum tree on DVE (8+4+2 adds) then two ones-matmuls
  accumulate the partition reduction into a [128,512] PSUM tile -- every
  partition holds the sum, so reciprocal+normalize are full-width 128-lane
  DVE ops (the f32r baseline burned 54us of PE on per-kt ones-matmuls and
  3.3us/head on single-lane [1,512] reciprocals). o_proj bf16, out stored
  bf16 (host upcasts and sums partials in f32).

Per-core PE streaming floor is ~335us (qkv 164 + scores/av 109 + ones 1.7 +
o_proj 55 + DR none); ACT exp floor ~141us fully overlapped.

HW exec time (8-core SPMD): see test.py output.
"""

import math
import os

import ml_dtypes
import numpy as np

import concourse.mybir as mybir
import concourse.tile as tile
from concourse import bacc
from concourse.bass_utils import run_bass_kernel_spmd

F32 = mybir.dt.float32
BF16 = mybir.dt.bfloat16
EXP = mybir.ActivationFunctionType.Exp

B, S, D = 2, 2048, 2048
H = 16
HD = 128
G = 4
HLOC = H // G          # 4 heads per core
DG = HLOC * HD         # 512
P = 128
NCORES = 8

DT = D // P            # 16 contraction tiles
GR = 512               # x granule (s columns)
NG = S // GR           # 4
SCHUNK = 512
QC = S // SCHUNK       # 4 query chunks
KT = S // P            # 16 key tiles
MT = DG // HD          # 4 stationary column blocks per projection
IC = D // SCHUNK       # 4
ST = GR // P           # 4
INV_SQRT_HD = 1.0 / math.sqrt(HD)

_cache = {}
last_run = None  # BassKernelResults of the most recent execution (for test.py)


def build():
    nc = bacc.Bacc(None, target_bir_lowering=False)

    xP_dr = nc.dram_tensor("xP", [NG * P, DT * GR], BF16, kind="ExternalInput")
    wqT_dr = nc.dram_tensor("wqT", [MT * P, DT * HD], BF16, kind="ExternalInput")
    wkT_dr = nc.dram_tensor("wkT", [MT * P, DT * HD], BF16, kind="ExternalInput")
    wvT_dr = nc.dram_tensor("wvT", [D, DG], BF16, kind="ExternalInput")
    woT_dr = nc.dram_tensor("woT", [DG, D], BF16, kind="ExternalInput")
    out_d = nc.dram_tensor("out", [S, D], BF16, kind="ExternalOutput")

    xP_v = xP_dr.rearrange("(g p) (o s) -> g p o s", p=P, s=GR)
    wqT_v = wqT_dr.rearrange("(m p) (o h) -> m p o h", p=P, h=HD)
    wkT_v = wkT_dr.rearrange("(m p) (o h) -> m p o h", p=P, h=HD)
    wvT_v = wvT_dr.rearrange("(o p) m -> p o m", p=P)
    woT_v = woT_dr.rearrange("(o p) i -> p o i", p=P)

    with tile.TileContext(nc) as tc:
        with (
            tc.tile_pool(name="persist", bufs=1) as persist,
            tc.tile_pool(name="wA", bufs=1) as wA,
            tc.tile_pool(name="xs", bufs=2) as xpool,
            tc.tile_pool(name="expp", bufs=4) as expp,
            tc.tile_pool(name="accp", bufs=4) as accp,
            tc.tile_pool(name="ctxp", bufs=2) as ctxp,
            tc.tile_pool(name="small", bufs=2) as small,
            tc.tile_pool(name="ps", bufs=1, space="PSUM") as psp,
        ):
            kTt = persist.tile([P, HLOC, S], BF16, tag="kT")
            qTt = persist.tile([P, HLOC, S], BF16, tag="qT")
            vvt = persist.tile([P, KT, DG], BF16, tag="vv")
            wot = persist.tile([P, MT, D], BF16, tag="wo")
            onesb = persist.tile([P, P], BF16, tag="ones")
            ones32 = persist.tile([P, P], F32, tag="ones32")
            nc.vector.memset(ones32[:], 1.0)
            nc.vector.tensor_copy(onesb[:], ones32[:])

            # ~4us of tiny matmuls bridging the initial DMA wait: keeps the
            # PE-HAM activity window busy so the real chains start at 2.4GHz
            # instead of paying ~3.4us of half-clock warmup mid-phase-A.
            # Lives in the pso tag, which is idle until phase B -- it must
            # NOT share rotation with the phase-A ps512 accumulators.
            warm = psp.tile([P, SCHUNK], F32, tag="pso", bufs=2)
            for _ in range(56):
                nc.tensor.matmul(warm[0:64, 0:64], onesb[:, 0:64],
                                 onesb[:, 0:64], start=True, stop=True)

            wk_sb = wA.tile([P, MT, DT, HD], BF16, tag="wk")
            wq_sb = wA.tile([P, MT, DT, HD], BF16, tag="wq")
            wv_sb = wA.tile([P, DT, DG], BF16, tag="wv")

            # weights on the ACT DMA ring (idle until phase B), x on sync.
            # First k-chain's weights sliced fine so PE starts early.
            xg0 = xpool.tile([P, DT, GR], BF16, tag="xg")
            for d0 in range(0, DT, 2):
                nc.scalar.dma_start(wk_sb[:, 0, d0:d0 + 2], wkT_v[0, :, d0:d0 + 2])
            # granule 0's upper half rides the scalar ring (one ring's issue
            # rate can't land 2MB before the first k chains drain it)
            for d0 in range(8, DT, 4):
                nc.scalar.dma_start(xg0[:, d0:d0 + 4], xP_v[0, :, d0:d0 + 4])
            for mt in range(1, MT):
                nc.scalar.dma_start(wk_sb[:, mt], wkT_v[mt])
            for d0 in range(0, DT, 4):
                nc.scalar.dma_start(wv_sb[:, d0:d0 + 4], wvT_v[:, d0:d0 + 4])
            for mt in range(MT):
                nc.scalar.dma_start(wq_sb[:, mt], wqT_v[mt])
            for jt in range(MT):
                nc.scalar.dma_start(wot[:, jt:jt + 1], woT_v[:, jt:jt + 1])

            # ---------- phase A: k, v, q projections ----------
            for g in range(NG):
                if g == 0:
                    xg = xg0
                    for d0 in range(4):
                        nc.sync.dma_start(xg[:, d0:d0 + 1], xP_v[g, :, d0:d0 + 1])
                    for d0 in range(4, 8, 2):
                        nc.sync.dma_start(xg[:, d0:d0 + 2], xP_v[g, :, d0:d0 + 2])
                else:
                    xg = xpool.tile([P, DT, GR], BF16, tag="xg")
                    for d0 in range(0, DT, 4):
                        nc.sync.dma_start(xg[:, d0:d0 + 4], xP_v[g, :, d0:d0 + 4])

                for mt in range(MT):  # k projection
                    ps = psp.tile([P, GR], F32, tag="ps512", bufs=2)
                    for dt in range(DT):
                        nc.tensor.matmul(
                            ps[:], wk_sb[:, mt, dt, :], xg[:, dt, :],
                            start=(dt == 0), stop=(dt == DT - 1))
                        if g == 0:
                            # no-dep filler: granule 0 is DMA-paced, and a
                            # >3.4us PE idle here re-throttles HAM to 1.2GHz
                            # for the next 14us (observed) -- keep it busy
                            nc.tensor.matmul(warm[0:64, 0:64],
                                             onesb[:, 0:64], onesb[:, 0:64],
                                             start=True, stop=True)
                    nc.vector.tensor_copy(
                        kTt[:, mt, g * GR:(g + 1) * GR], ps[:])

                for st in range(ST):  # v projection
                    ps = psp.tile([P, DG], F32, tag="ps512", bufs=2)
                    for dt in range(DT):
                        nc.tensor.matmul(
                            ps[:], xg[:, dt, st * P:(st + 1) * P], wv_sb[:, dt, :],
                            start=(dt == 0), stop=(dt == DT - 1))
                    nc.vector.tensor_copy(vvt[:, g * ST + st, :], ps[:])

                for mt in range(MT):  # q projection
                    ps = psp.tile([P, GR], F32, tag="ps512", bufs=2)
                    for dt in range(DT):
                        nc.tensor.matmul(
                            ps[:], wq_sb[:, mt, dt, :], xg[:, dt, :],
                            start=(dt == 0), stop=(dt == DT - 1))
                    nc.vector.tensor_copy(
                        qTt[:, mt, g * GR:(g + 1) * GR], ps[:])

            # ---------- phase B: attention + o_proj ----------
            def oproj_group(ctx_c, c, st, ic):
                stile = c * ST + st
                ps = psp.tile([P, SCHUNK], F32, tag="ps512", bufs=2)
                for jt in range(MT):
                    nc.tensor.matmul(
                        ps[:], ctx_c[:, jt, st * P:(st + 1) * P],
                        wot[:, jt, ic * SCHUNK:(ic + 1) * SCHUNK],
                        start=(jt == 0), stop=(jt == MT - 1))
                ob = small.tile([P, SCHUNK], BF16, tag="ostage", bufs=3)
                nc.vector.tensor_copy(ob[:], ps[:])
                # sync HWDGE ring is idle in phase B (x loads done) and
                # avoids the multi-us SWDGE drain at teardown
                nc.sync.dma_start(
                    out_d[stile * P:(stile + 1) * P,
                          ic * SCHUNK:(ic + 1) * SCHUNK],
                    ob[:])

            # o_proj of chunk c-1 is issued interleaved between the heads of
            # chunk c: its ps512 tiles then rotate between the psden tiles
            # instead of queueing behind all four of them, and its matmuls
            # give PE filler work at every head boundary
            pending = []  # (ctx, c, st, ic) groups not yet issued
            for c in range(QC):
                ctx = ctxp.tile([P, HLOC, SCHUNK], BF16, tag="ctx")
                for h in range(HLOC):
                    pso = psp.tile([P, SCHUNK], F32, tag="pso", bufs=2)
                    # denominator bf16 pair-sum tree on DVE, issued inline so
                    # tile reuse never waits on a later-issued consumer
                    l1, l2, l3 = [], [], []
                    for kp in range(KT // 2):
                        pss = psp.tile([P, 2, SCHUNK], F32, tag="pss", bufs=2)
                        for j in range(2):
                            kt = 2 * kp + j
                            nc.tensor.matmul(
                                pss[:, j, :],
                                kTt[:, h, kt * P:(kt + 1) * P],
                                qTt[:, h, c * SCHUNK:(c + 1) * SCHUNK],
                                start=True, stop=True)
                        ex = expp.tile([P, 2, SCHUNK], BF16, tag="expP",
                                       bufs=6)
                        nc.scalar.activation(ex[:], pss[:], EXP,
                                             scale=INV_SQRT_HD)
                        for j in range(2):
                            kt = 2 * kp + j
                            nc.tensor.matmul(
                                pso[:], vvt[:, kt, h * HD:(h + 1) * HD],
                                ex[:, j, :],
                                start=(kt == 0), stop=(kt == KT - 1))
                        t = accp.tile([P, SCHUNK], BF16, tag="pa1", bufs=4)
                        nc.vector.tensor_add(t[:], ex[:, 0, :], ex[:, 1, :])
                        l1.append(t)
                        if kp % 2 == 1:
                            t2 = accp.tile([P, SCHUNK], BF16, tag="pa2", bufs=4)
                            nc.vector.tensor_add(t2[:], l1[-2][:], l1[-1][:])
                            l2.append(t2)
                        if kp % 4 == 3:
                            t3 = accp.tile([P, SCHUNK], BF16, tag="pa3", bufs=2)
                            nc.vector.tensor_add(t3[:], l2[-2][:], l2[-1][:])
                            l3.append(t3)

                    # o_proj filler BEFORE the denominator wrap-up: the
                    # ones-matmuls wait ~1us on the DVE tree, and the
                    # in-order PE queue would sit idle at every head end
                    for _ in range(min(4, len(pending))):
                        oproj_group(*pending.pop(0))

                    # two ones-matmuls reduce over partitions into [128,512]
                    # (every partition = the sum -> no broadcast needed)
                    psden = psp.tile([P, SCHUNK], F32, tag="ps512", bufs=2)
                    for i, t in enumerate(l3):
                        nc.tensor.matmul(psden[:], onesb[:], t[:],
                                         start=(i == 0), stop=(i == len(l3) - 1))
                    rb = small.tile([P, SCHUNK], F32, tag="rb")
                    # ~51 ULP is plenty for a softmax denominator; the exact
                    # DVE reciprocal costs 3.4us/call and stalled the pso
                    # PSUM pipeline
                    nc.vector.reciprocal_approx_fast(rb[:], psden[:])
                    nc.vector.tensor_mul(ctx[:, h, :], pso[:], rb[:])

                for st in range(ST):
                    for ic in range(IC):
                        pending.append((ctx, c, st, ic))
            while pending:
                oproj_group(*pending.pop(0))

    nc.finalize()
    return nc


def _bf16(a):
    return np.asarray(a, dtype=np.float32).astype(ml_dtypes.bfloat16)


def _pack_x(xT):
    """[D, S] bf16 -> [NG*P, DT*GR] blocked so device DMAs are contiguous."""
    return np.ascontiguousarray(
        xT.reshape(DT, P, NG, GR).transpose(2, 1, 0, 3)
    ).reshape(NG * P, DT * GR)


def _pack_w_mt(wT):
    """[D, DG] bf16 -> [MT*P, DT*HD] blocked by stationary column block."""
    return np.ascontiguousarray(
        wT.reshape(DT, P, MT, HD).transpose(2, 1, 0, 3)
    ).reshape(MT * P, DT * HD)


def kernel(hidden_states, wq, wk, wv, wo):
    global last_run
    if "nc" not in _cache:
        _cache["nc"] = build()
    nc = _cache["nc"]

    hidden_states = np.asarray(hidden_states, dtype=np.float32)
    wq = np.asarray(wq, dtype=np.float32)
    wk = np.asarray(wk, dtype=np.float32)
    wv = np.asarray(wv, dtype=np.float32)
    wo = np.asarray(wo, dtype=np.float32)

    xP = [_pack_x(_bf16(hidden_states[b].T)) for b in range(B)]
    in_maps = []
    for c in range(NCORES):
        b, g = divmod(c, G)
        sl = slice(g * DG, (g + 1) * DG)
        in_maps.append({
            "xP": xP[b],
            "wqT": _pack_w_mt(_bf16(wq[sl, :].T)),
            "wkT": _pack_w_mt(_bf16(wk[sl, :].T)),
            "wvT": _bf16(wv[sl, :].T),
            "woT": _bf16(wo[:, sl].T),
        })

    trace = os.environ.get("BASSKERNEL_TRACE", "0") == "1"
    last_run = run_bass_kernel_spmd(
        nc, in_maps, core_ids=list(range(NCORES)), trace=trace)

    out = np.empty((B, S, D), dtype=np.float32)
    for b in range(B):
        acc = None
        for g in range(G):
            part = last_run.results[b * G + g]["out"].astype(np.float32)
            acc = part if acc is None else acc + part
        out[b] = acc
    return out
